# revision 22
# baseline (speedup 1.0000x reference)
"""Trainium2 Bass kernel for nn_MeanProbExtractor_yolov5 (NMS detection).

Full-input contract: kernel(YOLOoutput=[16,25200,85] f32) -> [16] f32.
Data-parallel over batch: 8 NeuronCores x 2 images each, SPMD.

v4 notes:
  - phase-A loads via SWDGE (spreads across all 16 SDMA engines); 3 chunks
    of ~67 anchors/partition with bufs=3 so tile recycle never stalls the
    DMA ring; rows 126-127 padded with duplicate data so DVE ops run on
    full 128 partitions (s rows stay -1 so no phantom candidates).
  - first chunk DMA issued before const setup (saves startup latency).
  - per-engine program order hand-interleaved across the two images:
    loads b0, consts, reduces b0, loads b1, tail-head b0, reduces b1 (c0,c1),
    pack/rows b0, reduce b1 c2, tail-head b1, A-build b0, fixpoint b0 (PE/
    scalar only) overlapping A-build b1 (vector), fixpoint b1.
  - fixpoint iterations are vector-free: u = inval + sum_jb k^T A[jb]
    (inval preloaded into PSUM, so no v-mask op), k = relu(sign(0.5-u)) on
    the scalar engine; A and k in bf16 (exact for 0/1 -> 4x faster PE).
  - count via accum_out of the final relu.
"""

import numpy as np

B_PER_CORE = 2
N_CORES = 8
N_ANCH = 25200
NFEAT = 85
TPP = 200  # anchors per partition; 126 * 200 = 25200 exactly
NP = 126  # partitions with real data
KCAP = 384  # compacted candidate slots (3 * 128); actual max 325
NBLK = KCAP // 128  # 3
SG_F = KCAP // 16  # sparse_gather output free size (24)
T_ITERS = 3
CONF_THRES = 0.25
LAM = float(np.float32(np.float32(0.45) / np.float32(1.45)))
CHUNKS = [(0, 100), (100, 100)]
CHMAX = 100

_CACHE = {}


def _build():
    import concourse.bass as bass
    import concourse.mybir as mybir
    import concourse.bacc as bacc
    import concourse.tile as tile
    from concourse.masks import make_identity

    f32 = mybir.dt.float32
    bf16 = mybir.dt.bfloat16
    i32 = mybir.dt.int32
    u32 = mybir.dt.uint32
    Alu = mybir.AluOpType
    Act = mybir.ActivationFunctionType
    X = mybir.AxisListType.X

    nc = bacc.Bacc("TRN2", target_bir_lowering=False, debug=False)

    xs = [
        nc.dram_tensor(f"x{b}", [N_ANCH, NFEAT], f32, kind="ExternalInput")
        for b in range(B_PER_CORE)
    ]
    out_dram = nc.dram_tensor("out", [1, B_PER_CORE], f32, kind="ExternalOutput")

    with tile.TileContext(nc) as tc:
        with (
            tc.tile_pool(name="const", bufs=1) as constp,
            tc.tile_pool(name="img", bufs=2) as imgp,
            tc.tile_pool(name="sA", bufs=2) as sap,
            tc.tile_pool(name="small", bufs=2) as smallp,
            tc.tile_pool(name="wrap", bufs=2) as wrapp,
            tc.tile_pool(name="rows", bufs=2) as rowsp,
            tc.tile_pool(name="rall", bufs=1) as rallp,
            tc.tile_pool(name="gath", bufs=2) as gathp,
            tc.tile_pool(name="amat", bufs=6) as amatp,
            tc.tile_pool(name="apers", bufs=2) as apersp,
            tc.tile_pool(name="krow", bufs=2) as krowp,
            tc.tile_pool(name="ps_tr", bufs=1, space="PSUM") as ps_trp,
            tc.tile_pool(name="ps_row", bufs=2, space="PSUM") as ps_rowp,
            tc.tile_pool(name="ps_u", bufs=2, space="PSUM") as ps_up,
        ):
            st = {}  # per-image state handles
            consts = {}

            def load_chunks(b):
                tiles = []
                for (c0, cl) in CHUNKS:
                    img = imgp.tile([128, CHMAX * NFEAT], f32, tag="img", name="img")
                    img3 = img[:].rearrange("p (t f) -> p t f", f=NFEAT)
                    nc.gpsimd.dma_start(
                        out=img3[0:NP, 0:cl, :],
                        in_=xs[b].ap()[:, :]
                        .rearrange("(p t) f -> p t f", t=TPP)[:, c0 : c0 + cl, :],
                    )
                    tiles.append((img3, c0, cl))
                st[b] = {"tiles": tiles}

            def reduce_chunk(b, ci):
                img3, c0, cl = st[b]["tiles"][ci]
                if ci == 0:
                    st[b]["mx"] = sap.tile([128, TPP], f32, tag="mx", name="mx")
                    st[b]["conf"] = sap.tile(
                        [128, TPP], f32, tag="conf", name="conf"
                    )
                    st[b]["ge"] = sap.tile([128, TPP], f32, tag="ge", name="ge")
                v3 = img3[0:NP, 0:cl, :]
                sl = slice(c0, c0 + cl)
                nc.vector.tensor_reduce(
                    out=st[b]["mx"][0:NP, sl], in_=v3[:, :, 5:NFEAT], axis=X,
                    op=Alu.max,
                )
                nc.vector.tensor_tensor(
                    out=st[b]["conf"][0:NP, sl], in0=v3[:, :, 4],
                    in1=st[b]["mx"][0:NP, sl], op=Alu.mult,
                )
                nc.vector.tensor_tensor(
                    out=st[b]["ge"][0:NP, sl], in0=v3[:, :, 5],
                    in1=st[b]["mx"][0:NP, sl], op=Alu.is_ge,
                )

            def finalize_s(b):
                conf, ge = st[b]["conf"], st[b]["ge"]
                # valid = (conf>T) & ge  (conf>T implies obj>T since mx<=1)
                m2 = sap.tile([128, TPP], f32, tag="m2")
                nc.vector.scalar_tensor_tensor(
                    out=m2[0:NP, :], in0=conf[0:NP, :], scalar=CONF_THRES,
                    in1=ge[0:NP, :], op0=Alu.is_gt, op1=Alu.mult,
                )
                s = sap.tile([128, TPP], f32, tag="s")
                nc.vector.memset(s[:], -1.0)
                nc.vector.scalar_tensor_tensor(
                    out=s[0:NP, :], in0=conf[0:NP, :], scalar=1.0,
                    in1=m2[0:NP, :], op0=Alu.add, op1=Alu.mult,
                )
                # s = conf where valid else -1 (rows >= NP stay -1)
                nc.vector.tensor_scalar(
                    s[0:NP, :], s[0:NP, :], 1.0, scalar2=None, op0=Alu.subtract
                )
                st[b]["s"] = s

            def tail_head(b):
                x = xs[b].ap()
                s = st[b]["s"]
                ident = consts["ident"]
                # ---- per-partition top-16 ----
                vals16 = smallp.tile([128, 16], f32, tag="vals16")
                idx16 = smallp.tile([128, 16], u32, tag="idx16")
                s2 = sap.tile([128, TPP], f32, tag="m2", name="s2")
                nc.vector.max(out=vals16[:, 0:8], in_=s[:])
                nc.vector.max_index(idx16[:, 0:8], vals16[:, 0:8], s[:])
                nc.vector.match_replace(
                    out=s2[:], in_to_replace=vals16[:, 0:8], in_values=s[:],
                    imm_value=-3.0,
                )
                nc.vector.max(out=vals16[:, 8:16], in_=s2[:])
                nc.vector.max_index(idx16[:, 8:16], vals16[:, 8:16], s2[:])

                # anchor index (or -1): anchm = (idx + p*TPP + 1)*(v>0) - 1
                idx16f = smallp.tile([128, 16], f32, tag="idx16f")
                nc.vector.tensor_copy(idx16f[:], idx16[:])
                anch1 = smallp.tile([128, 16], f32, tag="anch1")
                nc.vector.tensor_tensor(
                    out=anch1[:], in0=idx16f[:],
                    in1=consts["iota1f"][:].to_broadcast([128, 16]), op=Alu.add,
                )
                vm16 = smallp.tile([128, 16], f32, tag="vm16")
                nc.vector.tensor_scalar(
                    vm16[:], vals16[:], 0.0, scalar2=None, op0=Alu.is_gt
                )
                anchm = smallp.tile([128, 16], f32, tag="anchm")
                nc.vector.tensor_tensor(
                    out=anchm[:], in0=anch1[:], in1=vm16[:], op=Alu.mult
                )
                nc.vector.tensor_scalar(
                    anchm[:], anchm[:], 1.0, scalar2=None, op0=Alu.subtract
                )

                # ---- wrap via PE transpose + sparse compaction ----
                vT = ps_trp.tile([16, 128], f32, tag="wT")
                nc.tensor.transpose(out=vT[:], in_=vals16[:], identity=ident[:])
                v16w = wrapp.tile([16, 128], f32, tag="v16w")
                nc.scalar.copy(v16w[:], vT[:])
                aT = ps_trp.tile([16, 128], f32, tag="wT")
                nc.tensor.transpose(out=aT[:], in_=anchm[:], identity=ident[:])
                a16w = wrapp.tile([16, 128], f32, tag="a16w")
                nc.scalar.copy(a16w[:], aT[:])

                sg_s = wrapp.tile([16, SG_F], f32, tag="sg_s")
                sg_a = wrapp.tile([16, SG_F], f32, tag="sg_a")
                nf1 = wrapp.tile([1, 1], u32, tag="nf1")
                nf2 = wrapp.tile([1, 1], u32, tag="nf2")
                nc.gpsimd.sparse_gather(out=sg_s[:], in_=v16w[:], num_found=nf1[:])
                nc.gpsimd.sparse_gather(out=sg_a[:], in_=a16w[:], num_found=nf2[:])

                # [16,SG_F] -> col layout [128, NBLK]
                s_col0 = smallp.tile([128, NBLK], f32, tag="s_col0")
                a_col = smallp.tile([128, NBLK], f32, tag="a_col")
                nc.sync.dma_start(
                    out=s_col0[:], in_=sg_s[:].rearrange("q (h c) -> q h c", c=NBLK)
                )
                nc.scalar.dma_start(
                    out=a_col[:], in_=sg_a[:].rearrange("q (h c) -> q h c", c=NBLK)
                )
                # mask slots beyond num_found (hw writes garbage there);
                # only s_col needs it -- a_col garbage is clamped pre-gather
                nf_f = smallp.tile([1, 1], f32, tag="nf_f")
                nc.vector.tensor_copy(nf_f[:], nf1[:])
                nf_b = smallp.tile([128, 1], f32, tag="nf_b")
                nc.gpsimd.partition_broadcast(nf_b[:], nf_f[:])
                slotm = smallp.tile([128, NBLK], u32, tag="slotm")
                nc.vector.tensor_scalar(
                    slotm[:], consts["l_col"][:], nf_b[:], scalar2=None,
                    op0=Alu.is_lt,
                )
                s_col = smallp.tile([128, NBLK], f32, tag="s_col")
                nc.vector.memset(s_col[:], -1.0)
                nc.vector.copy_predicated(s_col[:], slotm[:], s_col0[:])
                a_int = smallp.tile([128, NBLK], i32, tag="a_int")
                nc.vector.tensor_copy(a_int[:], a_col[:])
                nc.vector.tensor_scalar(
                    a_int[:], a_int[:], 0, scalar2=None, op0=Alu.max
                )
                nc.vector.tensor_scalar(
                    a_int[:], a_int[:], N_ANCH - 1, scalar2=None, op0=Alu.min
                )

                # ---- gather candidate rows into one [128, 3*85] tile ----
                gc3 = gathp.tile([128, NBLK * NFEAT], f32, tag="gc3")
                for c in range(NBLK):
                    nc.gpsimd.indirect_dma_start(
                        out=gc3[:, c * NFEAT : (c + 1) * NFEAT],
                        out_offset=None,
                        in_=x,
                        in_offset=bass.IndirectOffsetOnAxis(
                            ap=a_int[:, c : c + 1], axis=0
                        ),
                    )
                st[b].update(s_col=s_col, gc3=gc3)

            def tail_mid_a(b):
                s_col, gc3 = st[b]["s_col"], st[b]["gc3"]
                ident = consts["ident"]
                g3 = gc3[:].rearrange("p (c f) -> p c f", f=NFEAT)
                # pack fields: 0:x1 1:y1 2:x2 3:y2 4:lam*area 5:s
                pack = smallp.tile([128, 18], f32, tag="pack")
                nc.vector.scalar_tensor_tensor(
                    out=pack[:, 0:NBLK], in0=g3[:, :, 2], scalar=-0.5,
                    in1=g3[:, :, 0], op0=Alu.mult, op1=Alu.add,
                )
                nc.vector.scalar_tensor_tensor(
                    out=pack[:, NBLK : 2 * NBLK], in0=g3[:, :, 3], scalar=-0.5,
                    in1=g3[:, :, 1], op0=Alu.mult, op1=Alu.add,
                )
                nc.vector.scalar_tensor_tensor(
                    out=pack[:, 2 * NBLK : 3 * NBLK], in0=g3[:, :, 2], scalar=0.5,
                    in1=g3[:, :, 0], op0=Alu.mult, op1=Alu.add,
                )
                nc.vector.scalar_tensor_tensor(
                    out=pack[:, 3 * NBLK : 4 * NBLK], in0=g3[:, :, 3], scalar=0.5,
                    in1=g3[:, :, 1], op0=Alu.mult, op1=Alu.add,
                )
                ax = smallp.tile([128, NBLK], f32, tag="ax")
                ay = smallp.tile([128, NBLK], f32, tag="ay")
                nc.vector.tensor_tensor(
                    out=ax[:], in0=pack[:, 2 * NBLK : 3 * NBLK],
                    in1=pack[:, 0:NBLK], op=Alu.subtract,
                )
                nc.vector.tensor_tensor(
                    out=ay[:], in0=pack[:, 3 * NBLK : 4 * NBLK],
                    in1=pack[:, NBLK : 2 * NBLK], op=Alu.subtract,
                )
                axl = smallp.tile([128, NBLK], f32, tag="axl")
                nc.vector.tensor_scalar(
                    axl[:], ax[:], LAM, scalar2=None, op0=Alu.mult
                )
                nc.vector.tensor_tensor(
                    out=pack[:, 4 * NBLK : 5 * NBLK], in0=axl[:], in1=ay[:],
                    op=Alu.mult,
                )
                nc.vector.tensor_copy(pack[:, 5 * NBLK : 6 * NBLK], s_col[:])

                # ---- transpose + one row-extraction DMA ----
                tr_ps = ps_trp.tile([18, 128], f32, tag="tr")
                nc.tensor.transpose(out=tr_ps[:], in_=pack[:], identity=ident[:])
                tr_sb = smallp.tile([18, 128], f32, tag="tr_sb")
                nc.scalar.copy(tr_sb[:], tr_ps[:])
                row_all = rallp.tile([1, 6 * KCAP], f32, tag="row_all")
                nc.sync.dma_start(
                    out=row_all[:].rearrange("o (r k) -> o r k", r=18),
                    in_=tr_sb[:],
                )

                # ---- broadcast rows [1,384] -> [128,384] via matmul ----
                rows_sb = []
                for f in range(6):
                    rp = ps_rowp.tile([128, KCAP], f32, tag="rowmat")
                    nc.tensor.matmul(
                        out=rp[:], lhsT=consts["ones_row"][:],
                        rhs=row_all[:, f * KCAP : (f + 1) * KCAP],
                        start=True, stop=True,
                    )
                    rsb = rowsp.tile([128, KCAP], f32, tag=f"row{f}")
                    nc.scalar.copy(rsb[:], rp[:])
                    rows_sb.append(rsb)

                # row views for fixpoint/readout
                inval_row = krowp.tile([1, KCAP], bf16, tag="inval_row")
                nc.vector.tensor_scalar(
                    inval_row[:], row_all[:, 5 * KCAP : 6 * KCAP], 0.0,
                    scalar2=None, op0=Alu.is_lt,
                )
                sp_row = krowp.tile([1, KCAP], f32, tag="sp_row")
                nc.vector.tensor_scalar(
                    sp_row[:], row_all[:, 5 * KCAP : 6 * KCAP], 0.0,
                    scalar2=None, op0=Alu.max,
                )
                st[b].update(
                    pack=pack, rows_sb=rows_sb,
                    inval_row=inval_row, sp_row=sp_row,
                )

            def tail_mid_b(b):
                pack = st[b]["pack"]
                x1r, y1r, x2r, y2r, ar, sr = st[b]["rows_sb"]
                s_col = st[b]["s_col"]
                Ab = []
                for blk in range(NBLK):
                    col = lambda f: pack[:, f * NBLK + blk : f * NBLK + blk + 1]
                    xx1 = amatp.tile([128, KCAP], f32, tag="scr")
                    nc.vector.tensor_scalar(
                        xx1[:], x1r[:], col(0), scalar2=None, op0=Alu.max
                    )
                    w = amatp.tile([128, KCAP], f32, tag="scr")
                    nc.vector.scalar_tensor_tensor(
                        out=w[:], in0=x2r[:], scalar=col(2), in1=xx1[:],
                        op0=Alu.min, op1=Alu.subtract,
                    )
                    yy1 = amatp.tile([128, KCAP], f32, tag="scr")
                    nc.vector.tensor_scalar(
                        yy1[:], y1r[:], col(1), scalar2=None, op0=Alu.max
                    )
                    h = amatp.tile([128, KCAP], f32, tag="scr")
                    nc.vector.scalar_tensor_tensor(
                        out=h[:], in0=y2r[:], scalar=col(3), in1=yy1[:],
                        op0=Alu.min, op1=Alu.subtract,
                    )
                    nc.scalar.activation(w[:], w[:], Act.Relu)
                    nc.scalar.activation(h[:], h[:], Act.Relu)
                    inter = amatp.tile([128, KCAP], f32, tag="scr")
                    nc.vector.tensor_tensor(
                        out=inter[:], in0=w[:], in1=h[:], op=Alu.mult
                    )
                    E = amatp.tile([128, KCAP], f32, tag="scr")
                    nc.vector.scalar_tensor_tensor(
                        out=E[:], in0=ar[:], scalar=col(4), in1=inter[:],
                        op0=Alu.add, op1=Alu.is_lt,
                    )
                    A = apersp.tile([128, KCAP], bf16, tag=f"A{blk}")
                    nc.vector.scalar_tensor_tensor(
                        out=A[:], in0=sr[:], scalar=col(5), in1=E[:],
                        op0=Alu.is_lt, op1=Alu.mult,
                    )
                    Ab.append(A)
                v_col = smallp.tile([128, NBLK], bf16, tag="v_col")
                nc.vector.tensor_scalar(
                    v_col[:], s_col[:], 0.0, scalar2=None, op0=Alu.is_gt
                )
                st[b].update(Ab=Ab, v_col=v_col)

            def tail_fix(b):
                Ab, v_col = st[b]["Ab"], st[b]["v_col"]
                inval_row, sp_row = st[b]["inval_row"], st[b]["sp_row"]
                ident = consts["ident"]
                k_col = v_col
                k_row = None
                cnt = krowp.tile([1, 1], f32, tag="cnt")
                for it in range(T_ITERS):
                    u_ps = ps_up.tile([1, KCAP], f32, tag="u")
                    nc.tensor.matmul(
                        out=u_ps[:], lhsT=consts["ones_bf"][:],
                        rhs=inval_row[:], start=True, stop=False,
                    )
                    for jb in range(NBLK):
                        nc.tensor.matmul(
                            out=u_ps[:],
                            lhsT=k_col[:, jb : jb + 1],
                            rhs=Ab[jb][:],
                            start=False,
                            stop=(jb == NBLK - 1),
                        )
                    # k = sigmoid(50*(0.5-u)): exactly 1.0 for u=0, ~1e-11
                    # for u>=1 (noise provably below all thresholds)
                    if it == T_ITERS - 1:
                        k_row = krowp.tile([1, KCAP], f32, tag="k_row")
                        nc.scalar.activation(
                            k_row[:], u_ps[:], Act.Sigmoid,
                            bias=consts["b25_c"][:], scale=consts["s50_c"][:],
                            accum_out=cnt[:],
                        )
                    else:
                        k_rb = krowp.tile([1, KCAP], f32, tag="k_rb")
                        nc.scalar.activation(
                            k_rb[:], u_ps[:], Act.Sigmoid,
                            bias=consts["b25_c"][:], scale=consts["s50_c"][:],
                        )
                        kT = ps_trp.tile([128, NBLK], f32, tag="kT")
                        for c in range(NBLK):
                            nc.tensor.transpose(
                                out=kT[:, c : c + 1],
                                in_=k_rb[:, c * 128 : (c + 1) * 128],
                                identity=consts["ident"][0:1, 0:1],
                            )
                        k_col = smallp.tile([128, NBLK], bf16, tag="k_col")
                        nc.scalar.copy(k_col[:], kT[:])

                # ---- readout ----
                ks_row = krowp.tile([1, KCAP], f32, tag="ks_row")
                nc.vector.tensor_tensor(
                    out=ks_row[:], in0=k_row[:], in1=sp_row[:], op=Alu.mult
                )
                ws = krowp.tile([1, 1], f32, tag="ws")
                nc.vector.tensor_reduce(out=ws[:], in_=ks_row[:], axis=X, op=Alu.add)
                d = krowp.tile([1, 1], f32, tag="d")
                nc.vector.tensor_scalar(
                    d[:], cnt[:], 1.0, scalar2=None, op0=Alu.max
                )
                r = krowp.tile([1, 1], f32, tag="r")
                nc.vector.reciprocal(r[:], d[:])
                res = krowp.tile([1, 1], f32, tag="res")
                nc.vector.tensor_tensor(
                    out=res[:], in0=ws[:], in1=r[:], op=Alu.mult
                )
                nc.sync.dma_start(out=out_dram.ap()[:, b : b + 1], in_=res[:])

            # ================= program order =================
            load_chunks(0)

            # ---- shared constants (after first loads: hides startup) ----
            ident = constp.tile([128, 128], f32)
            make_identity(nc, ident[:])
            ones_row = constp.tile([1, 128], f32)
            nc.vector.memset(ones_row[:], 1.0)
            iota1 = constp.tile([128, 1], i32)
            nc.gpsimd.iota(iota1[:], pattern=[[0, 1]], base=1, channel_multiplier=TPP)
            iota1f = constp.tile([128, 1], f32)
            nc.vector.tensor_copy(iota1f[:], iota1[:])
            # sparse-stream order index l for each col-layout slot:
            # slot (p, c) has l = 16*(3*(p%8)+c) + p//8
            lw_i = constp.tile([16, SG_F], i32)
            nc.gpsimd.iota(lw_i[:], pattern=[[16, SG_F]], base=0, channel_multiplier=1)
            lw_f = constp.tile([16, SG_F], f32)
            nc.vector.tensor_copy(lw_f[:], lw_i[:])
            l_col = constp.tile([128, NBLK], f32)
            nc.sync.dma_start(
                out=l_col[:], in_=lw_f[:].rearrange("q (h c) -> q h c", c=NBLK)
            )
            b25_c = constp.tile([1, 1], f32)
            nc.vector.memset(b25_c[:], 25.0)
            s50_c = constp.tile([1, 1], f32)
            nc.vector.memset(s50_c[:], -50.0)
            ones_bf = constp.tile([1, 1], bf16)
            nc.vector.memset(ones_bf[:], 1.0)
            consts.update(
                ident=ident, ones_row=ones_row, iota1f=iota1f, l_col=l_col,
                b25_c=b25_c, s50_c=s50_c, ones_bf=ones_bf,
            )

            reduce_chunk(0, 0)
            reduce_chunk(0, 1)
            finalize_s(0)
            load_chunks(1)
            reduce_chunk(1, 0)
            reduce_chunk(1, 1)
            finalize_s(1)
            tail_head(0)
            tail_mid_a(0)
            tail_mid_b(0)
            tail_fix(0)
            tail_head(1)
            tail_mid_a(1)
            tail_mid_b(1)
            tail_fix(1)

    nc.compile()
    return nc


def _get_nc():
    if "nc" not in _CACHE:
        _CACHE["nc"] = _build()
    return _CACHE["nc"]


def kernel(YOLOoutput: np.ndarray) -> np.ndarray:
    from concourse.bass_utils import run_bass_kernel_spmd

    x = np.ascontiguousarray(np.asarray(YOLOoutput, dtype=np.float32))
    assert x.shape == (N_CORES * B_PER_CORE, N_ANCH, NFEAT)
    nc = _get_nc()
    in_maps = [
        {
            f"x{b}": np.ascontiguousarray(x[i * B_PER_CORE + b])
            for b in range(B_PER_CORE)
        }
        for i in range(N_CORES)
    ]
    res = run_bass_kernel_spmd(nc, in_maps, core_ids=list(range(N_CORES)))
    out = np.concatenate([r["out"].reshape(B_PER_CORE) for r in res.results])
    return out.astype(np.float32)
